# revision 40
# baseline (speedup 1.0000x reference)
"""Windowed (Swin-style) multi-head attention on 8 TRN2 NeuronCores.

Data-parallel: 256 independent windows -> 32 per core. Per window:
  qkv = x @ w_qkv ; per-head attn = softmax(q k^T * scale + bias) ; out = (attn v) @ w_proj + b_proj

Device-side layout strategy (all matmuls contract over the partition dim):
  - host pre-transposes x to channel-major xT[c, tok] so qT/kT are produced
    feature-major (ready to be score-matmul operands) and v token-major.
  - scores are computed TRANSPOSED, S^T[k, q] (lhsT = kT tile, rhs = qT), so
    softmax normalization runs over the partition axis:
      exp via ScalarE (scale folded in), * exp(bias) via VectorE,
      column-sums via ones-block matmul on TensorE (32 replicated rows at
      32-aligned partitions), reciprocal_approx_fast on VectorE, broadcast
      back to feature rows via an indicator matmul.
  - avT[f, q] = v-contracted matmul accumulated over k tiles; normalized
    avT is exactly the lhsT the projection matmul needs. b_proj is added
    (pre-broadcast on host) during the PSUM->SBUF output copy.
Matmul operands are bf16 (full-rate PE, fp32 PSUM accumulation); inputs are
rounded to bf16 on the host so they can be DMA'd directly.
"""

import sys

for _p in ("/opt/trn_rl_repo",):
    if _p not in sys.path:
        sys.path.insert(0, _p)

import ml_dtypes
import numpy as np
from contextlib import ExitStack

import concourse.bass as bass
import concourse.bacc as bacc
import concourse.mybir as mybir
from concourse import tile
from concourse.bass_utils import run_bass_kernel_spmd

NCORES = 8
BS = 256
W = BS // NCORES  # windows per core
N = 256           # tokens per window
DIM = 512
NH = 8
HD = 64
SCALE = HD ** -0.5
F32 = mybir.dt.float32
BF16 = mybir.dt.bfloat16
NPBF = ml_dtypes.bfloat16
EXP = mybir.ActivationFunctionType.Exp
COPY = mybir.ActivationFunctionType.Copy


def build(w_count=W):
    nc = bacc.Bacc(None, target_bir_lowering=False)
    xt = nc.declare_dram_parameter("xt", [w_count, DIM, N], BF16, False)
    wqk = nc.declare_dram_parameter("wqk", [DIM, 2 * DIM], BF16, False)
    wv = nc.declare_dram_parameter("wv", [DIM, DIM], BF16, False)
    wp = nc.declare_dram_parameter("wp", [DIM, DIM], BF16, False)
    brep = nc.declare_dram_parameter("brep", [128, DIM], F32, False)
    ebt = nc.declare_dram_parameter("ebt", [128, 2 * NH * N], BF16, False)
    ones_c = nc.declare_dram_parameter("ones_c", [128, HD], BF16, False)
    out = nc.declare_dram_parameter("out", [w_count, N, DIM], F32, True)

    with ExitStack() as ctx:
        tc = ctx.enter_context(tile.TileContext(nc))
        const = ctx.enter_context(tc.tile_pool(name="const", bufs=1))
        p_xt = ctx.enter_context(tc.tile_pool(name="xt", bufs=4))
        p_qk = ctx.enter_context(tc.tile_pool(name="qk", bufs=3))
        p_v = ctx.enter_context(tc.tile_pool(name="v", bufs=3))
        p_p = ctx.enter_context(tc.tile_pool(name="pp", bufs=4))
        p_e = ctx.enter_context(tc.tile_pool(name="te", bufs=6))
        p_bc = ctx.enter_context(tc.tile_pool(name="bc", bufs=4))
        p_av = ctx.enter_context(tc.tile_pool(name="av", bufs=4))
        p_rs = ctx.enter_context(tc.tile_pool(name="rs", bufs=3))
        p_o = ctx.enter_context(tc.tile_pool(name="os", bufs=6))
        ps = ctx.enter_context(tc.tile_pool(name="ps", bufs=2, space="PSUM"))
        psp = ctx.enter_context(tc.tile_pool(name="psp", bufs=2, space="PSUM"))
        ps2 = ctx.enter_context(tc.tile_pool(name="ps2", bufs=2, space="PSUM"))

        # wqk + ones on the sync queue (ahead of the xT loads, so the first
        # qkT can start ASAP); the later-needed constants go via gpsimd so
        # they never delay the xT stream.
        # wqk split per c-tile so the very first qkT matmul only waits for
        # one quarter of it; later-needed constants go via gpsimd (ebt
        # first: scores consume it before the now-deferred v phase).
        wqk_s = const.tile([128, 4, 2 * DIM], BF16)
        wqk_r = wqk.ap().rearrange("(t p) f -> p t f", p=128)
        # wqk rides the scalar-hosted queue in parallel with xt on sync;
        # pair 0 below runs ct-major so compute starts once ct0 arrives;
        # ct0 split in half so the first ft-group matmuls start sooner.
        nc.scalar.dma_start(wqk_s[:, 0, 0:DIM], wqk_r[:, 0, 0:DIM])
        nc.scalar.dma_start(wqk_s[:, 0, DIM:], wqk_r[:, 0, DIM:])
        for ct in range(1, 4):
            nc.scalar.dma_start(wqk_s[:, ct, :], wqk_r[:, ct, :])

        xt_ap0 = xt.ap()
        xt0 = p_xt.tile([128, 4, 2, N], BF16, tag="xt", name="xt_boot")
        for ct in range(4):
            nc.sync.dma_start(xt0[:, ct, 0, :],
                              xt_ap0[0, 128 * ct:128 * (ct + 1), :])
            nc.gpsimd.dma_start(xt0[:, ct, 1, :],
                                xt_ap0[1, 128 * ct:128 * (ct + 1), :])
        oc_s = const.tile([128, HD], BF16)
        nc.gpsimd.dma_start(oc_s[:], ones_c.ap())
        eb_s = const.tile([128, 2 * NH * N], BF16)
        nc.gpsimd.dma_start(eb_s[:], ebt.ap())
        wv_s = const.tile([128, 4, DIM], BF16)
        nc.gpsimd.dma_start(wv_s[:], wv.ap().rearrange("(t p) f -> p t f", p=128))
        wp_s = const.tile([128, 4, DIM], BF16)
        nc.gpsimd.dma_start(wp_s[:], wp.ap().rearrange("(t p) f -> p t f", p=128))
        br_s = const.tile([128, DIM], F32)
        nc.gpsimd.dma_start(br_s[:], brep.ap())

        xt_ap = xt.ap()
        out_ap = out.ap()

        # pp column index for (head, ktile): per head-pair the layout is
        # (e_k0, e_k1, o_k0, o_k1); score matmuls are ISSUED interleaved
        # e_k0, o_k0, e_k1, o_k1 so adjacent matmuls hit disjoint PE row
        # groups (and rowsum/avT orderings hit disjoint col groups).
        def ppi(h, kt):
            return (h // 2) * 4 + (h % 2) * 2 + kt

        for wp2 in range(w_count // 2):
            w0 = 2 * wp2
            # load xT (channel-major) for both windows: [128, ct, win, tok];
            # first pair split per-ct so the first qkv matmul only waits for
            # ct0, with the remaining wqk c-tiles interleaved between
            if wp2 == 0:
                xt_s = xt0  # loaded up front across two queues
            else:
                xt_s = p_xt.tile([128, 4, 2, N], BF16, tag="xt")
                for wl in range(2):
                    nc.sync.dma_start(
                        xt_s[:, :, wl, :],
                        xt_ap[w0 + wl].rearrange("(t p) q -> p t q", p=128),
                    )

            # qkT[feat, (win tok)] batched over the window pair (N=512 keeps
            # LDWEIGHTS hidden behind the matmul)
            qk_s = p_qk.tile([128, 8, 2, N], BF16, tag="qk")
            if wp2 == 0:
                # boot pair runs ct-major with 8 concurrent PSUM groups
                # (borrowing the idle scores/rowsum pools) so the first
                # matmul only waits for the ct0 DMAs
                accs = [ps.tile([128, 512], F32, tag="ps", name=f"qb_{f}")
                        for f in range(2)]
                accs += [psp.tile([128, 512], F32, tag="scp", name=f"qb_{f}")
                         for f in range(2, 4)]
                accs += [ps2.tile([128, 512], F32, tag="rs2", name=f"qb_{f}")
                         for f in range(4, 6)]
                for ct in range(4):
                    for ft in range(6):
                        nc.tensor.matmul(
                            accs[ft][:],
                            wqk_s[:, ct, ft * 128:(ft + 1) * 128],
                            xt_s[:, ct, :, :],
                            start=(ct == 0),
                            stop=(ct == 3),
                        )
                for ft in range(6):
                    nc.scalar.activation(
                        qk_s[:, ft, :, :].rearrange("p a q -> p (a q)"),
                        accs[ft][:], COPY,
                    )
                for ft in range(6, 8):
                    acc = ps.tile([128, 512], F32, tag="ps")
                    for ct in range(4):
                        nc.tensor.matmul(
                            acc[:],
                            wqk_s[:, ct, ft * 128:(ft + 1) * 128],
                            xt_s[:, ct, :, :],
                            start=(ct == 0),
                            stop=(ct == 3),
                        )
                    nc.scalar.activation(
                        qk_s[:, ft, :, :].rearrange("p a q -> p (a q)"),
                        acc[:], COPY,
                    )
            else:
                for ft in range(8):
                    acc = ps.tile([128, 512], F32, tag="ps")
                    for ct in range(4):
                        nc.tensor.matmul(
                            acc[:],
                            wqk_s[:, ct, ft * 128:(ft + 1) * 128],
                            xt_s[:, ct, :, :],
                            start=(ct == 0),
                            stop=(ct == 3),
                        )
                    nc.scalar.activation(
                        qk_s[:, ft, :, :].rearrange("p a q -> p (a q)"), acc[:], COPY
                    )

            # v[tok, feat] (token-major), per window; emitted AFTER the
            # score phases (v is first consumed at avT, so this keeps the
            # qkT -> scores critical path short)
            v_s = p_v.tile([128, 2, 2, DIM], BF16, tag="v")

            def phase_v():
                for wi in range(2):
                    for kt in range(2):
                        acc = ps.tile([128, 512], F32, tag="ps")
                        for ct in range(4):
                            nc.tensor.matmul(
                                acc[:],
                                xt_s[:, ct, wi, kt * 128:(kt + 1) * 128],
                                wv_s[:, ct, :],
                                start=(ct == 0),
                                stop=(ct == 3),
                            )
                        nc.vector.tensor_copy(
                            v_s[:, wi, kt, :], acc[:]
                        )

            # phases per window, emitted interleaved (S0 S1 V R0 A0 P0 R1
            # A1 P1) so each in-order engine always has ready work queued
            # ahead of any dependency-stalled op.
            def phase_scores(wi):
                # scores^T -> exp(scale*s) * exp(bias) -> pp; then rowsums
                pp_s = p_p.tile([128, 2 * NH * N], BF16, tag="pp",
                                name=f"pp_{wi}")
                for hp in range(4):
                    scpE = psp.tile([128, 2, N], F32, tag="scp")
                    scpO = psp.tile([128, 2, N], F32, tag="scp")
                    scp = [scpE, scpO]
                    for kt in range(2):
                        for s in range(2):
                            h = 2 * hp + s
                            base = s * HD
                            nc.tensor.matmul(
                                scp[s][:, kt, :],
                                qk_s[base:base + HD, 4 + hp, wi,
                                     kt * 128:(kt + 1) * 128],
                                qk_s[base:base + HD, hp, wi, :],
                                start=True,
                                stop=True,
                            )
                    for s in range(2):
                        te = p_e.tile([128, 2 * N], BF16, tag="te",
                                      name=f"te_{wi}_{hp}_{s}")
                        nc.scalar.activation(
                            te[:], scp[s][:].rearrange("p a q -> p (a q)"),
                            EXP, scale=SCALE,
                        )
                        off = (hp * 4 + s * 2) * N
                        nc.vector.tensor_mul(
                            pp_s[:, off:off + 2 * N],
                            te[:],
                            eb_s[:, off:off + 2 * N],
                        )
                # rowsums broadcast straight to feature rows: ones-block
                # M=64 matmuls write head (2*ftl+s)'s denominator onto the
                # 64 partitions its avT features occupy; one 1-bank tile per
                # feature-tile pair.
                rs0 = ps2.tile([128, 2, N], F32, tag="rs2", name=f"rs0_{wi}")
                rs1 = ps2.tile([128, 2, N], F32, tag="rs2", name=f"rs1_{wi}")
                rsp = [rs0, rs1]
                for ap2 in range(2):
                    for sub2 in range(2):
                        ftl = 2 * ap2 + sub2
                        for s in range(2):
                            h = 2 * ftl + s
                            for kt in range(2):
                                nc.tensor.matmul(
                                    rsp[ap2][s * HD:(s + 1) * HD, sub2, :],
                                    oc_s[:],
                                    pp_s[:, ppi(h, kt) * N:(ppi(h, kt) + 1) * N],
                                    start=(kt == 0),
                                    stop=(kt == 1),
                                    tile_position=(0, s * HD),
                                )
                return pp_s, rsp

            def phase_recip(wi, rsp):
                # reciprocal (fp32 fast-approx) per feature-tile pair;
                # result is directly the avT normalizer
                rb0 = p_rs.tile([128, 512], F32, tag="rb", name=f"rb0_{wi}")
                rb1 = p_rs.tile([128, 512], F32, tag="rb", name=f"rb1_{wi}")
                rbs = [rb0, rb1]
                for ap2 in range(2):
                    nc.vector.reciprocal_approx_fast(
                        rbs[ap2][:], rsp[ap2][:].rearrange("p a q -> p (a q)")
                    )
                return rbs

            def phase_av(wi, pp_s, rbs):
                # avT[f, q]: head pairs in PE column groups; batched
                # normalize-mult per two feature tiles
                av_s = p_av.tile([128, 4 * N], BF16, tag="av",
                                 name=f"av_{wi}")
                for ap2 in range(2):
                    aa = ps.tile([128, 512], F32, tag="work",
                                 name=f"aa_{wi}_{ap2}")
                    for sub2 in range(2):
                        ftl = 2 * ap2 + sub2
                        for sub in range(2):
                            h = 2 * ftl + sub
                            for kt in range(2):
                                nc.tensor.matmul(
                                    aa[sub * HD:(sub + 1) * HD,
                                       sub2 * N:(sub2 + 1) * N],
                                    v_s[:, wi, kt, h * HD:(h + 1) * HD],
                                    pp_s[:, ppi(h, kt) * N:(ppi(h, kt) + 1) * N],
                                    start=(kt == 0),
                                    stop=(kt == 1),
                                    tile_position=(0, sub * HD),
                                )
                    nc.vector.tensor_mul(
                        av_s[:, ap2 * 512:(ap2 + 1) * 512],
                        aa[:],
                        rbs[ap2][:],
                    )
                return av_s

            def phase_proj(wi, av_s):
                # projection; add b_proj during PSUM->SBUF copy; output DMA
                # on the gpsimd queue (keeps the sync queue free for loads)
                w = w0 + wi
                for qt in range(2):
                    oo = ps.tile([128, 512], F32, tag="work",
                                 name=f"oo_{wi}_{qt}")
                    for ftl in range(4):
                        nc.tensor.matmul(
                            oo[:],
                            av_s[:, ftl * N + qt * 128:ftl * N + qt * 128 + 128],
                            wp_s[:, ftl, :],
                            start=(ftl == 0),
                            stop=(ftl == 3),
                        )
                    o_s = p_o.tile([128, DIM], F32, tag="os",
                                   name=f"os_{wi}_{qt}")
                    nc.vector.tensor_add(o_s[:], oo[:], br_s[:])
                    nc.gpsimd.dma_start(
                        out_ap[w, qt * 128:(qt + 1) * 128, :], o_s[:]
                    )

            pp0, rsp0 = phase_scores(0)
            pp1, rsp1 = phase_scores(1)
            phase_v()
            rb0 = phase_recip(0, rsp0)
            av0 = phase_av(0, pp0, rb0)
            phase_proj(0, av0)
            rb1 = phase_recip(1, rsp1)
            av1 = phase_av(1, pp1, rb1)
            phase_proj(1, av1)

    nc.finalize()
    return nc


_NC_CACHE = {}


def _get_nc(w_count):
    if w_count not in _NC_CACHE:
        _NC_CACHE[w_count] = build(w_count)
    return _NC_CACHE[w_count]


def _prep(inputs, w_count):
    x = np.asarray(inputs["x"], dtype=np.float32)
    noise = np.asarray(inputs["noise"], dtype=np.float32)
    ns = np.asarray(inputs["noise_strength"], dtype=np.float32)
    wqkv = np.asarray(inputs["w_qkv"], dtype=np.float32)
    wproj = np.asarray(inputs["w_proj"], dtype=np.float32)
    bproj = np.asarray(inputs["b_proj"], dtype=np.float32)
    bt = np.asarray(inputs["bias_table"], dtype=np.float32)
    ri = np.asarray(inputs["rel_index"])

    xe = x + noise * ns                                     # [BS, N, DIM]
    xt = np.ascontiguousarray(xe.transpose(0, 2, 1).astype(NPBF))
    eb = np.exp(bt[ri])                                     # [q, k, h]
    ebT = eb.transpose(2, 1, 0)                             # [h, k, q]
    ebt = np.zeros((128, 2 * NH, N), np.float32)
    for h in range(NH):
        for kt in range(2):
            i = (h // 2) * 4 + (h % 2) * 2 + kt
            ebt[:, i, :] = ebT[h, kt * 128:(kt + 1) * 128, :]
    ebt = np.ascontiguousarray(ebt.reshape(128, 2 * NH * N).astype(NPBF))
    common = {
        "wqk": np.ascontiguousarray(wqkv[:, : 2 * DIM].astype(NPBF)),
        "wv": np.ascontiguousarray(wqkv[:, 2 * DIM:].astype(NPBF)),
        "wp": np.ascontiguousarray(wproj.astype(NPBF)),
        "brep": np.ascontiguousarray(
            np.broadcast_to(bproj.reshape(1, DIM), (128, DIM)).astype(np.float32)
        ),
        "ebt": ebt,
        "ones_c": np.ones((128, HD), NPBF),
    }
    in_maps = []
    for i in range(NCORES):
        m = dict(common)
        m["xt"] = np.ascontiguousarray(xt[i * w_count:(i + 1) * w_count])
        in_maps.append(m)
    return in_maps


def _run(inputs, w_count=W, trace=False, trace_cores=None):
    nc = _get_nc(w_count)
    in_maps = _prep(inputs, w_count)
    kw = {}
    if trace_cores is not None:
        kw["trace_cores"] = trace_cores
    res = run_bass_kernel_spmd(
        nc, in_maps, core_ids=list(range(NCORES)), trace=trace, **kw
    )
    full = np.concatenate([res.results[i]["out"] for i in range(NCORES)], axis=0)
    return full, res


def kernel(**inputs):
    out, _ = _run(inputs, W, trace=False)
    return out


def kernel_profiled(inputs, w_count=W, trace_cores=None):
    out, res = _run(inputs, w_count, trace=True, trace_cores=trace_cores)
    return out, res

